# revision 9
# baseline (speedup 1.0000x reference)
"""v4: pair-symmetry kernel, rebalanced across engines.

Same math as v2 (w(p,s)*|dsal(p,s)| symmetric under (p,s)->(p+s,-s); each
of the 60 shift pairs computed once on an extended domain and accumulated
twice into PSUM), with:
  - mirror (re-shifted) accumulation done directly per pair with
    shifted-identity matmuls (lhsT = partition-offset view of the identity)
    instead of the psG -> SBUF -> row-shift-DMA -> psA chain;
  - pairs processed two at a time through shared double-wide tmp tiles so
    Square/Exp/channel-adds/abs/P-mul are one instruction per two pairs;
  - cho (odd-parity shifted copy) built with SBUF->SBUF DMA, not ACT;
  - erosion ladder, boundary memsets and the final reductions moved to the
    Pool engine; loss_map evacuation on ACT.

Layout: 120 partitions x 3 payload rows (global row 3p-6+j), per-channel
local window 13 rows x 372 cols fp16, all 4 channels in one tile.
"""

import numpy as np

H = W = 352
RADIUS = 5
NP = 120                 # partitions; payload rows 3p-6 .. 3p-4
PADW2 = W + 20           # 372 : cols idx t <-> global col t-10
LROWS = 13               # local rows k <-> global row 3p-11+k
CH = LROWS * PADW2       # 4836 elements per channel
PW = W + 2 * RADIUS      # 362 : P/ssq domain, col q <-> global col q-5
N_CORES = 8

_CACHE = {}


def _build_kernel():
    from contextlib import ExitStack

    import concourse.bass as bass
    import concourse.tile as tile
    from concourse import bacc, mybir

    f16 = mybir.dt.float16
    f32 = mybir.dt.float32
    i16 = mybir.dt.int16
    Alu = mybir.AluOpType
    Act = mybir.ActivationFunctionType

    nc = bacc.Bacc(
        "TRN2",
        debug=False,
        enable_asserts=False,
        target_bir_lowering=False,
        num_devices=1,
        enable_partition_id=False,
    )
    # host-padded fp16 inputs: row r <-> global row r-11, col t <-> global t-10
    pred_d = nc.dram_tensor("pred16", [370, PADW2], f16, kind="ExternalInput")
    feat_d = nc.dram_tensor("feat16", [3, 370, PADW2], f16, kind="ExternalInput")
    out_d = nc.dram_tensor("partial", [NP, 2], f32, kind="ExternalOutput")

    with tile.TileContext(nc) as tc, ExitStack() as ctx:
        persist = ctx.enter_context(tc.tile_pool(name="persist", bufs=1))

        # all 4 channels in one tile; odd-shifted copy of the rgb channels
        ch4 = persist.tile([NP, 4, LROWS, PADW2], f16, tag="ch4")
        cho = persist.tile([NP, 3, LROWS, PADW2], f16, tag="cho")

        # sal (c=3) first so the mask pipeline overlaps the rgb loads
        for c in (3, 0, 1, 2):
            src_ap = pred_d.ap() if c == 3 else feat_d.ap()[c]
            src = bass.AP(
                tensor=src_ap.tensor,
                offset=src_ap.offset,
                ap=[[3 * PADW2, NP], [PADW2, LROWS], [1, PADW2]],
            )
            nc.sync.dma_start(out=ch4[:, c, :, :], in_=src)
        ch4f = ch4[:].rearrange("p c a b -> p (c a b)")
        chof = cho[:].rearrange("p c a b -> p (c a b)")
        # odd-parity copy via SBUF->SBUF DMA (off the compute engines)
        for c in range(3):
            nc.sync.dma_start(
                out=chof[:, c * CH : (c + 1) * CH],
                in_=ch4f[:, c * CH + 1 : (c + 1) * CH + 1],
            )

        zeros = persist.tile([1, 5 * PADW2], f16, tag="zeros")
        nc.gpsimd.memset(zeros[:], 0.0)

        # ---- contour mask (in P-column coords, [NP, 3, 362]) ----
        sal = ch4[:, 3]
        salf = sal.rearrange("p a b -> p (a b)")
        with tc.tile_pool(name="maskpool", bufs=1) as mp:
            lbl = mp.tile([NP, LROWS, PADW2], f16, tag="lbl")
            nc.vector.tensor_scalar(
                out=lbl.rearrange("p a b -> p (a b)"), in0=salf,
                scalar1=0.5, scalar2=None, op0=Alu.is_gt,
            )
            u = mp.tile([NP, LROWS, PADW2], f16, tag="u")
            nc.vector.tensor_scalar(
                out=u.rearrange("p a b -> p (a b)"), in0=salf,
                scalar1=0.5, scalar2=None, op0=Alu.is_le,
            )
            # invalidate u outside the image: pad cols, then pad rows the
            # +-2 pool windows can reach (k in [3,10])
            nc.gpsimd.memset(u[:, :, 0:10], 0.0)
            nc.gpsimd.memset(u[:, :, 362:372], 0.0)
            nc.gpsimd.memset(u[0:1, 3:11, :], 0.0)
            nc.sync.dma_start(out=u[1:2, 3:8, :], in_=zeros[:, 0 : 5 * PADW2])
            nc.sync.dma_start(out=u[2:3, 3:5, :], in_=zeros[:, 0 : 2 * PADW2])
            nc.sync.dma_start(out=u[118:119, 9:11, :], in_=zeros[:, 0 : 2 * PADW2])
            nc.sync.dma_start(out=u[119:120, 6:11, :], in_=zeros[:, 0 : 5 * PADW2])

            ladA = mp.tile([NP, 6, PADW2], f16, tag="ladA")
            ladB = mp.tile([NP, 3, PADW2], f16, tag="ladB")
            rowm = mp.tile([NP, 3, PADW2], f16, tag="rowm")
            ladC = mp.tile([NP, 3, PW + 2], f16, tag="ladC")
            ladD = mp.tile([NP, 3, PW], f16, tag="ladD")
            ladA2 = mp.tile([NP, 6, PADW2], f16, tag="ladA2")
            ladB2 = mp.tile([NP, 3, PADW2], f16, tag="ladB2")
            rowm2 = mp.tile([NP, 3, PADW2], f16, tag="rowm2")
            ladC2 = mp.tile([NP, 3, PW + 2], f16, tag="ladC2")
            ladD2 = mp.tile([NP, 3, PW], f16, tag="ladD2")
            dil = persist.tile([NP, 3, PW], f16, tag="dil")
            umax = persist.tile([NP, 3, PW], f16, tag="umax")
            # dilation ladder on DVE, erosion ladder on Pool
            for eng, srcb, dstb, lA, lB, rm, lC, lD in (
                (nc.vector, lbl, dil, ladA, ladB, rowm, ladC, ladD),
                (nc.gpsimd, u, umax, ladA2, ladB2, rowm2, ladC2, ladD2),
            ):
                eng.tensor_max(lA[:], srcb[:, 3:9, :], srcb[:, 4:10, :])
                eng.tensor_max(lB[:], lA[:, 0:3, :], lA[:, 2:5, :])
                eng.tensor_max(rm[:], lB[:], srcb[:, 7:10, :])
                # cols: dil[q] = max rowm[t=q+3..q+7]
                eng.tensor_max(lC[:], rm[:, :, 3 : 3 + PW + 2], rm[:, :, 4 : 4 + PW + 2])
                eng.tensor_max(lD[:], lC[:, :, 0:PW], lC[:, :, 2 : 2 + PW])
                eng.tensor_max(dstb[:], lD[:], rm[:, :, 7 : 7 + PW])
        mask = persist.tile([NP, 3, PW], f16, tag="mask")
        nc.vector.scalar_tensor_tensor(
            out=mask[:].rearrange("p a b -> p (a b)"),
            in0=dil[:].rearrange("p a b -> p (a b)"),
            scalar=-1.0,
            in1=umax[:].rearrange("p a b -> p (a b)"),
            op0=Alu.add, op1=Alu.add,
        )
        # zero mask outside the image: pad cols, pad partitions, junk rows
        nc.gpsimd.memset(mask[:, :, 0:RADIUS], 0.0)
        nc.gpsimd.memset(mask[:, :, RADIUS + W : PW], 0.0)
        nc.gpsimd.memset(mask[0:2, :, :], 0.0)
        nc.sync.dma_start(out=mask[119:120, 1:3, :], in_=zeros[:, 0 : 2 * PW])

        sums = persist.tile([NP, 2], f32, tag="sums")
        nc.gpsimd.memset(sums[:], 0.0)
        # mask sum early (Pool, full reduce to one scalar), overlaps the
        # pair loop; partitions 1.. stay zero
        nc.gpsimd.tensor_reduce(
            out=sums[0:1, 1:2], in_=mask[:].rearrange("p a b -> p (a b)"),
            axis=mybir.AxisListType.XYZWC, op=Alu.add,
        )

        # ---- identity + shifted identities for PE accumulation ----
        # ident_shift[a][k, m] = 1 iff m == k + a  (out[m] += P[m - a])
        rowidx = persist.tile([NP, NP], i16, tag="rowidx")
        pidx = persist.tile([NP, 1], mybir.dt.int32, tag="pidx")
        pidxf = persist.tile([NP, 1], f32, tag="pidxf")
        nc.gpsimd.iota(rowidx[:], pattern=[[1, NP]], base=0, channel_multiplier=0)
        nc.gpsimd.iota(pidx[:], pattern=[[1, 1]], base=0, channel_multiplier=1)
        nc.vector.tensor_copy(out=pidxf[:], in_=pidx[:])
        ident_shift = []
        for a in range(3):
            ida = persist.tile([NP, NP], f16, tag=f"ident{a}", name=f"ident{a}")
            pa = pidxf
            if a > 0:
                pa = persist.tile([NP, 1], f32, tag=f"pidxf{a}", name=f"pidxf{a}")
                nc.vector.tensor_scalar(
                    out=pa[:], in0=pidxf[:], scalar1=float(a), scalar2=None,
                    op0=Alu.add,
                )
            nc.vector.tensor_scalar(
                out=ida[:], in0=rowidx[:], scalar1=pa[:], scalar2=None,
                op0=Alu.is_equal,
            )
            ident_shift.append(ida)
        ident = ident_shift[0]

        pp = ctx.enter_context(tc.tile_pool(name="ps", bufs=1, space="PSUM"))
        psA = pp.tile([NP, 3, 512], f32, tag="psA")

        tmp = ctx.enter_context(tc.tile_pool(name="tmp", bufs=3))

        # half set: sy>0 all sx; sy=0 positive sx
        pairs = [(sy, sx) for sy in range(5, 0, -1) for sx in range(-5, 6)]
        pairs += [(0, sx) for sx in range(1, 6)]
        chunks = [pairs[i : i + 2] for i in range(0, len(pairs), 2)]

        n_mm = 0

        def acc(out, lhsT, rhs, last):
            nonlocal n_mm
            # first pair's direct j=0,1,2 reset the three psA regions
            nc.tensor.matmul(
                out=out, lhsT=lhsT, rhs=rhs,
                start=(n_mm < 3), stop=last,
                skip_group_check=True,
            )
            n_mm += 1

        total_mm = 6 * len(pairs)

        for ci, chunk in enumerate(chunks):
            k = len(chunk)
            # pair-ch fused double tiles: channels of pair i at 3i..3i+2
            d2 = tmp.tile([NP, 3 * k, 3, PW], f16, tag="d2")
            ssq2 = tmp.tile([NP, k, 3, PW], f16, tag="ssq2")
            wgt2 = tmp.tile([NP, k, 3, PW], f16, tag="wgt2")
            dsal2 = tmp.tile([NP, k, 3, PW], f16, tag="dsal2")
            adsal2 = tmp.tile([NP, k, 3, PW], f16, tag="adsal2")
            P2 = tmp.tile([NP, k, 3, PW], f16, tag="P2")

            for i, (sy, sx) in enumerate(chunk):
                off_par = (5 + sx) % 2
                if off_par == 0:
                    winr = ch4[:, 0:3, 5 + sy : 8 + sy, 5 + sx : 5 + sx + PW]
                else:
                    winr = cho[:, 0:3, 5 + sy : 8 + sy, 4 + sx : 4 + sx + PW]
                ctr = cho[:, 0:3, 5:8, 4 : 4 + PW]
                nc.vector.tensor_sub(d2[:, 3 * i : 3 * i + 3, :, :], winr, ctr)
                nc.gpsimd.tensor_sub(
                    dsal2[:, i, :, :],
                    ch4[:, 3, 5 + sy : 8 + sy, 5 + sx : 5 + sx + PW],
                    ch4[:, 3, 5:8, 5 : 5 + PW],
                )

            d2f = d2[:].rearrange("p c a b -> p (c a b)")
            # squares in place over the diffs (one ACT instr per chunk)
            nc.scalar.activation(out=d2f, in_=d2f, func=Act.Square)
            # channel sums: ssq = q0 + q1; ssq += q2  (strided pair views)
            nk = 3 * k
            q0 = d2[:, 0:nk:3, :, :]
            q1 = d2[:, 1:nk:3, :, :]
            q2v = d2[:, 2:nk:3, :, :]
            nc.vector.tensor_add(ssq2[:], q0, q1)
            nc.vector.tensor_add(ssq2[:], ssq2[:], q2v)
            nc.scalar.activation(
                out=wgt2[:].rearrange("p c a b -> p (c a b)"),
                in_=ssq2[:].rearrange("p c a b -> p (c a b)"),
                func=Act.Exp, scale=-200.0,
            )
            nc.vector.tensor_scalar(
                out=adsal2[:].rearrange("p c a b -> p (c a b)").bitcast(mybir.dt.uint16),
                in0=dsal2[:].rearrange("p c a b -> p (c a b)").bitcast(mybir.dt.uint16),
                scalar1=0x7FFF, scalar2=None, op0=Alu.bitwise_and,
            )
            nc.vector.tensor_mul(
                P2[:].rearrange("p c a b -> p (c a b)"),
                wgt2[:].rearrange("p c a b -> p (c a b)"),
                adsal2[:].rearrange("p c a b -> p (c a b)"),
            )

            # PE accumulation: direct (shared ident) plus mirrors, ordered by
            # lhsT partition shift so identical weights stay loaded
            mms = []  # (a, out, lhsT, rhs)
            for i, (sy, sx) in enumerate(chunk):
                for j in range(3):
                    mms.append((0, psA[:, j, 0:PW], ident[:], P2[:, i, j, :]))
            for i, (sy, sx) in enumerate(chunk):
                if sy == 0:
                    for j in range(3):
                        mms.append((0, psA[:, j, RADIUS : RADIUS + W], ident[:],
                                    P2[:, i, j, RADIUS - sx : RADIUS - sx + W]))
                else:
                    # row-shift by sy via shifted-identity:
                    # psA[p, j] += P[p + dp, jp]; dp = (j - sy - jp)/3 <= 0
                    for j in range(3):
                        jp = (j - sy) % 3
                        dp = (j - sy - jp) // 3
                        a = -dp  # superdiagonal offset
                        mms.append((a, psA[:, j, RADIUS : RADIUS + W],
                                    ident_shift[a][:],
                                    P2[:, i, jp, RADIUS - sx : RADIUS - sx + W]))
            mms.sort(key=lambda t: t[0])
            for _, out_ap, lhsT, rhs in mms:
                acc(out_ap, lhsT, rhs, n_mm == total_mm - 1)

        lm = persist.tile([NP, 3, PW], f16, tag="lm")
        nc.scalar.copy(out=lm[:], in_=psA[:, :, 0:PW])

        # ---- masked partial sum (fused multiply + reduce) ----
        scratch = persist.tile([NP, 3, PW], f16, tag="scratch")
        nc.vector.tensor_tensor_reduce(
            out=scratch[:].rearrange("p a b -> p (a b)"),
            in0=lm[:].rearrange("p a b -> p (a b)"),
            in1=mask[:].rearrange("p a b -> p (a b)"),
            scale=1.0,
            scalar=0.0,
            op0=Alu.mult, op1=Alu.add,
            accum_out=sums[:, 0:1],
        )
        nc.sync.dma_start(out=out_d.ap(), in_=sums[:])

    nc.compile()
    return nc


def kernel(pred, feat):
    import os

    # A stale PJRT compilation-cache hit was observed to return a bad
    # executable (NaN result); force a fresh compile per process.
    os.environ.setdefault("JAX_ENABLE_COMPILATION_CACHE", "false")
    try:
        import jax

        jax.config.update("jax_enable_compilation_cache", False)
    except Exception:
        pass

    if "nc" not in _CACHE:
        _CACHE["nc"] = _build_kernel()
    nc = _CACHE["nc"]
    from concourse.bass_utils import run_bass_kernel_spmd

    pred = np.asarray(pred, dtype=np.float32).reshape(N_CORES, H, W)
    feat = np.asarray(feat, dtype=np.float32).reshape(N_CORES, 3, H, W)
    predp = np.zeros((N_CORES, 370, PADW2), np.float16)
    predp[:, 11:363, 10:362] = pred.astype(np.float16)
    featp = np.zeros((N_CORES, 3, 370, PADW2), np.float16)
    featp[:, :, 11:363, 10:362] = feat.astype(np.float16)
    in_maps = [
        {"pred16": np.ascontiguousarray(predp[i]),
         "feat16": np.ascontiguousarray(featp[i])}
        for i in range(N_CORES)
    ]
    res = run_bass_kernel_spmd(nc, in_maps, core_ids=list(range(N_CORES)))
    _CACHE["last_results"] = res
    tot = np.zeros(2, np.float64)
    for r in res.results:
        tot += r["partial"].astype(np.float64).sum(axis=0)
    loss = tot[0] / (tot[1] + 1e-6)
    return np.array(loss, dtype=np.float32)


# revision 13
# speedup vs baseline: 1.0193x; 1.0193x over previous
"""v4: pair-symmetry kernel, rebalanced across engines.

Same math as v2 (w(p,s)*|dsal(p,s)| symmetric under (p,s)->(p+s,-s); each
of the 60 shift pairs computed once on an extended domain and accumulated
twice into PSUM), with:
  - mirror (re-shifted) accumulation done directly per pair with
    shifted-identity matmuls (lhsT = partition-offset view of the identity)
    instead of the psG -> SBUF -> row-shift-DMA -> psA chain;
  - pairs processed two at a time through shared double-wide tmp tiles so
    Square/Exp/channel-adds/abs/P-mul are one instruction per two pairs;
  - cho (odd-parity shifted copy) built with SBUF->SBUF DMA, not ACT;
  - erosion ladder, boundary memsets and the final reductions moved to the
    Pool engine; loss_map evacuation on ACT.

Layout: 120 partitions x 3 payload rows (global row 3p-6+j), per-channel
local window 13 rows x 372 cols fp16, all 4 channels in one tile.
"""

import numpy as np

H = W = 352
RADIUS = 5
NP = 120                 # partitions; payload rows 3p-6 .. 3p-4
PADW2 = W + 20           # 372 : cols idx t <-> global col t-10
LROWS = 13               # local rows k <-> global row 3p-11+k
CH = LROWS * PADW2       # 4836 elements per channel
PW = W + 2 * RADIUS      # 362 : P/ssq domain, col q <-> global col q-5
N_CORES = 8

_CACHE = {}


def _build_kernel():
    from contextlib import ExitStack

    import concourse.bass as bass
    import concourse.tile as tile
    from concourse import bacc, mybir

    f16 = mybir.dt.float16
    f32 = mybir.dt.float32
    i16 = mybir.dt.int16
    Alu = mybir.AluOpType
    Act = mybir.ActivationFunctionType

    nc = bacc.Bacc(
        "TRN2",
        debug=False,
        enable_asserts=False,
        target_bir_lowering=False,
        num_devices=1,
        enable_partition_id=False,
    )
    # host-padded fp16 inputs: row r <-> global row r-11, col t <-> global t-10
    pred_d = nc.dram_tensor("pred16", [370, PADW2], f16, kind="ExternalInput")
    feat_d = nc.dram_tensor("feat16", [3, 370, PADW2], f16, kind="ExternalInput")
    out_d = nc.dram_tensor("partial", [NP, 2], f32, kind="ExternalOutput")

    with tile.TileContext(nc) as tc, ExitStack() as ctx:
        persist = ctx.enter_context(tc.tile_pool(name="persist", bufs=1))

        # all 4 channels in one tile; odd-shifted copy of the rgb channels
        ch4 = persist.tile([NP, 4, LROWS, PADW2], f16, tag="ch4")
        cho = persist.tile([NP, 3, LROWS, PADW2], f16, tag="cho")

        # sal (c=3) first so the mask pipeline overlaps the rgb loads
        for c in (3, 0, 1, 2):
            src_ap = pred_d.ap() if c == 3 else feat_d.ap()[c]
            src = bass.AP(
                tensor=src_ap.tensor,
                offset=src_ap.offset,
                ap=[[3 * PADW2, NP], [PADW2, LROWS], [1, PADW2]],
            )
            nc.sync.dma_start(out=ch4[:, c, :, :], in_=src)
        ch4f = ch4[:].rearrange("p c a b -> p (c a b)")
        chof = cho[:].rearrange("p c a b -> p (c a b)")
        # odd-parity copy via SBUF->SBUF DMA (off the compute engines)
        for c in range(3):
            nc.sync.dma_start(
                out=chof[:, c * CH : (c + 1) * CH],
                in_=ch4f[:, c * CH + 1 : (c + 1) * CH + 1],
            )

        zeros = persist.tile([1, 5 * PADW2], f16, tag="zeros")
        nc.gpsimd.memset(zeros[:], 0.0)

        # ---- contour mask (in P-column coords, [NP, 3, 362]) ----
        sal = ch4[:, 3]
        salf = sal.rearrange("p a b -> p (a b)")
        with tc.tile_pool(name="maskpool", bufs=1) as mp:
            lbl = mp.tile([NP, LROWS, PADW2], f16, tag="lbl")
            nc.vector.tensor_scalar(
                out=lbl.rearrange("p a b -> p (a b)"), in0=salf,
                scalar1=0.5, scalar2=None, op0=Alu.is_gt,
            )
            u = mp.tile([NP, LROWS, PADW2], f16, tag="u")
            nc.vector.tensor_scalar(
                out=u.rearrange("p a b -> p (a b)"), in0=salf,
                scalar1=0.5, scalar2=None, op0=Alu.is_le,
            )
            # invalidate u outside the image: pad cols, then pad rows the
            # +-2 pool windows can reach (k in [3,10])
            nc.gpsimd.memset(u[:, :, 0:10], 0.0)
            nc.gpsimd.memset(u[:, :, 362:372], 0.0)
            nc.gpsimd.memset(u[0:1, 3:11, :], 0.0)
            nc.sync.dma_start(out=u[1:2, 3:8, :], in_=zeros[:, 0 : 5 * PADW2])
            nc.sync.dma_start(out=u[2:3, 3:5, :], in_=zeros[:, 0 : 2 * PADW2])
            nc.sync.dma_start(out=u[118:119, 9:11, :], in_=zeros[:, 0 : 2 * PADW2])
            nc.sync.dma_start(out=u[119:120, 6:11, :], in_=zeros[:, 0 : 5 * PADW2])

            ladA = mp.tile([NP, 6, PADW2], f16, tag="ladA")
            ladB = mp.tile([NP, 3, PADW2], f16, tag="ladB")
            rowm = mp.tile([NP, 3, PADW2], f16, tag="rowm")
            ladC = mp.tile([NP, 3, PW + 2], f16, tag="ladC")
            ladD = mp.tile([NP, 3, PW], f16, tag="ladD")
            ladA2 = mp.tile([NP, 6, PADW2], f16, tag="ladA2")
            ladB2 = mp.tile([NP, 3, PADW2], f16, tag="ladB2")
            rowm2 = mp.tile([NP, 3, PADW2], f16, tag="rowm2")
            ladC2 = mp.tile([NP, 3, PW + 2], f16, tag="ladC2")
            ladD2 = mp.tile([NP, 3, PW], f16, tag="ladD2")
            dil = persist.tile([NP, 3, PW], f16, tag="dil")
            umax = persist.tile([NP, 3, PW], f16, tag="umax")
            # dilation ladder on DVE, erosion ladder on Pool
            for eng, srcb, dstb, lA, lB, rm, lC, lD in (
                (nc.vector, lbl, dil, ladA, ladB, rowm, ladC, ladD),
                (nc.gpsimd, u, umax, ladA2, ladB2, rowm2, ladC2, ladD2),
            ):
                eng.tensor_max(lA[:], srcb[:, 3:9, :], srcb[:, 4:10, :])
                eng.tensor_max(lB[:], lA[:, 0:3, :], lA[:, 2:5, :])
                eng.tensor_max(rm[:], lB[:], srcb[:, 7:10, :])
                # cols: dil[q] = max rowm[t=q+3..q+7]
                eng.tensor_max(lC[:], rm[:, :, 3 : 3 + PW + 2], rm[:, :, 4 : 4 + PW + 2])
                eng.tensor_max(lD[:], lC[:, :, 0:PW], lC[:, :, 2 : 2 + PW])
                eng.tensor_max(dstb[:], lD[:], rm[:, :, 7 : 7 + PW])
        mask = persist.tile([NP, 3, PW], f16, tag="mask")
        nc.vector.scalar_tensor_tensor(
            out=mask[:].rearrange("p a b -> p (a b)"),
            in0=dil[:].rearrange("p a b -> p (a b)"),
            scalar=-1.0,
            in1=umax[:].rearrange("p a b -> p (a b)"),
            op0=Alu.add, op1=Alu.add,
        )
        # zero mask outside the image: pad cols, pad partitions, junk rows
        nc.gpsimd.memset(mask[:, :, 0:RADIUS], 0.0)
        nc.gpsimd.memset(mask[:, :, RADIUS + W : PW], 0.0)
        nc.gpsimd.memset(mask[0:2, :, :], 0.0)
        nc.sync.dma_start(out=mask[119:120, 1:3, :], in_=zeros[:, 0 : 2 * PW])

        sums = persist.tile([NP, 2], f32, tag="sums")
        nc.gpsimd.memset(sums[:], 0.0)
        # mask sum early (Pool, full reduce to one scalar), overlaps the
        # pair loop; partitions 1.. stay zero
        nc.gpsimd.tensor_reduce(
            out=sums[0:1, 1:2], in_=mask[:].rearrange("p a b -> p (a b)"),
            axis=mybir.AxisListType.XYZWC, op=Alu.add,
        )

        # ---- identity + shifted identities for PE accumulation ----
        # ident_shift[a][k, m] = 1 iff m == k + a  (out[m] += P[m - a])
        rowidx = persist.tile([NP, NP], i16, tag="rowidx")
        pidx = persist.tile([NP, 1], mybir.dt.int32, tag="pidx")
        pidxf = persist.tile([NP, 1], f32, tag="pidxf")
        nc.gpsimd.iota(rowidx[:], pattern=[[1, NP]], base=0, channel_multiplier=0)
        nc.gpsimd.iota(pidx[:], pattern=[[1, 1]], base=0, channel_multiplier=1)
        nc.vector.tensor_copy(out=pidxf[:], in_=pidx[:])
        ident_shift = []
        for a in range(3):
            ida = persist.tile([NP, NP], f16, tag=f"ident{a}", name=f"ident{a}")
            pa = pidxf
            if a > 0:
                pa = persist.tile([NP, 1], f32, tag=f"pidxf{a}", name=f"pidxf{a}")
                nc.vector.tensor_scalar(
                    out=pa[:], in0=pidxf[:], scalar1=float(a), scalar2=None,
                    op0=Alu.add,
                )
            nc.vector.tensor_scalar(
                out=ida[:], in0=rowidx[:], scalar1=pa[:], scalar2=None,
                op0=Alu.is_equal,
            )
            ident_shift.append(ida)
        ident = ident_shift[0]

        pp = ctx.enter_context(tc.tile_pool(name="ps", bufs=1, space="PSUM"))
        psA = pp.tile([NP, 3, 512], f32, tag="psA")

        tmp = ctx.enter_context(tc.tile_pool(name="tmp", bufs=5))

        # half set: sy>0 all sx; sy=0 positive sx
        pairs = [(sy, sx) for sy in range(5, 0, -1) for sx in range(-5, 6)]
        pairs += [(0, sx) for sx in range(1, 6)]
        chunks = [pairs[i : i + 2] for i in range(0, len(pairs), 2)]

        n_mm = 0

        def acc(out, lhsT, rhs, last):
            nonlocal n_mm
            # first pair's direct j=0,1,2 reset the three psA regions
            nc.tensor.matmul(
                out=out, lhsT=lhsT, rhs=rhs,
                start=(n_mm < 3), stop=last,
                skip_group_check=True,
            )
            n_mm += 1

        total_mm = 6 * len(pairs)

        for ci, chunk in enumerate(chunks):
            k = len(chunk)
            # pair-ch fused double tiles: channels of pair i at 3i..3i+2
            d2 = tmp.tile([NP, 3 * k, 3, PW], f16, tag="d2")
            wgt2 = tmp.tile([NP, k, 3, PW], f16, tag="wgt2")
            dsal2 = tmp.tile([NP, k, 3, PW], f16, tag="dsal2")

            for i, (sy, sx) in enumerate(chunk):
                off_par = (5 + sx) % 2
                if off_par == 0:
                    winr = ch4[:, 0:3, 5 + sy : 8 + sy, 5 + sx : 5 + sx + PW]
                else:
                    winr = cho[:, 0:3, 5 + sy : 8 + sy, 4 + sx : 4 + sx + PW]
                ctr = cho[:, 0:3, 5:8, 4 : 4 + PW]
                nc.vector.tensor_sub(d2[:, 3 * i : 3 * i + 3, :, :], winr, ctr)
                nc.gpsimd.tensor_sub(
                    dsal2[:, i, :, :],
                    ch4[:, 3, 5 + sy : 8 + sy, 5 + sx : 5 + sx + PW],
                    ch4[:, 3, 5:8, 5 : 5 + PW],
                )

            d2f = d2[:].rearrange("p c a b -> p (c a b)")
            # squares in place over the diffs (one ACT instr per chunk)
            nc.scalar.activation(out=d2f, in_=d2f, func=Act.Square)
            # channel sums accumulated into the q0 slice of d2
            nk = 3 * k
            q0 = d2[:, 0:nk:3, :, :]
            q1 = d2[:, 1:nk:3, :, :]
            q2v = d2[:, 2:nk:3, :, :]
            nc.vector.tensor_add(q0, q0, q1)
            nc.vector.tensor_add(q0, q0, q2v)
            nc.scalar.activation(
                out=wgt2[:], in_=q0, func=Act.Exp, scale=-200.0,
            )
            # |dsal| in place, then P = wgt * |dsal| over wgt2
            dsal2f = dsal2[:].rearrange("p c a b -> p (c a b)")
            nc.vector.tensor_scalar(
                out=dsal2f.bitcast(mybir.dt.uint16),
                in0=dsal2f.bitcast(mybir.dt.uint16),
                scalar1=0x7FFF, scalar2=None, op0=Alu.bitwise_and,
            )
            nc.vector.tensor_mul(
                wgt2[:].rearrange("p c a b -> p (c a b)"),
                wgt2[:].rearrange("p c a b -> p (c a b)"),
                dsal2f,
            )
            P2 = wgt2

            # PE accumulation: direct (shared ident) plus mirrors, ordered by
            # lhsT partition shift so identical weights stay loaded
            mms = []  # (a, out, lhsT, rhs)
            for i, (sy, sx) in enumerate(chunk):
                for j in range(3):
                    mms.append((0, psA[:, j, 0:PW], ident[:], P2[:, i, j, :]))
            for i, (sy, sx) in enumerate(chunk):
                if sy == 0:
                    for j in range(3):
                        mms.append((0, psA[:, j, RADIUS : RADIUS + W], ident[:],
                                    P2[:, i, j, RADIUS - sx : RADIUS - sx + W]))
                else:
                    # row-shift by sy via shifted-identity:
                    # psA[p, j] += P[p + dp, jp]; dp = (j - sy - jp)/3 <= 0
                    for j in range(3):
                        jp = (j - sy) % 3
                        dp = (j - sy - jp) // 3
                        a = -dp  # superdiagonal offset
                        mms.append((a, psA[:, j, RADIUS : RADIUS + W],
                                    ident_shift[a][:],
                                    P2[:, i, jp, RADIUS - sx : RADIUS - sx + W]))
            mms.sort(key=lambda t: t[0])
            for _, out_ap, lhsT, rhs in mms:
                acc(out_ap, lhsT, rhs, n_mm == total_mm - 1)

        lm = persist.tile([NP, 3, PW], f16, tag="lm")
        nc.scalar.copy(out=lm[:], in_=psA[:, :, 0:PW])

        # ---- masked partial sum (fused multiply + reduce) ----
        scratch = persist.tile([NP, 3, PW], f16, tag="scratch")
        nc.vector.tensor_tensor_reduce(
            out=scratch[:].rearrange("p a b -> p (a b)"),
            in0=lm[:].rearrange("p a b -> p (a b)"),
            in1=mask[:].rearrange("p a b -> p (a b)"),
            scale=1.0,
            scalar=0.0,
            op0=Alu.mult, op1=Alu.add,
            accum_out=sums[:, 0:1],
        )
        nc.sync.dma_start(out=out_d.ap(), in_=sums[:])

    nc.compile()
    return nc


def kernel(pred, feat):
    import os

    # A stale PJRT compilation-cache hit was observed to return a bad
    # executable (NaN result); force a fresh compile per process.
    os.environ.setdefault("JAX_ENABLE_COMPILATION_CACHE", "false")
    try:
        import jax

        jax.config.update("jax_enable_compilation_cache", False)
    except Exception:
        pass

    if "nc" not in _CACHE:
        _CACHE["nc"] = _build_kernel()
    nc = _CACHE["nc"]
    from concourse.bass_utils import run_bass_kernel_spmd

    pred = np.asarray(pred, dtype=np.float32).reshape(N_CORES, H, W)
    feat = np.asarray(feat, dtype=np.float32).reshape(N_CORES, 3, H, W)
    predp = np.zeros((N_CORES, 370, PADW2), np.float16)
    predp[:, 11:363, 10:362] = pred.astype(np.float16)
    featp = np.zeros((N_CORES, 3, 370, PADW2), np.float16)
    featp[:, :, 11:363, 10:362] = feat.astype(np.float16)
    in_maps = [
        {"pred16": np.ascontiguousarray(predp[i]),
         "feat16": np.ascontiguousarray(featp[i])}
        for i in range(N_CORES)
    ]
    res = run_bass_kernel_spmd(nc, in_maps, core_ids=list(range(N_CORES)))
    _CACHE["last_results"] = res
    tot = np.zeros(2, np.float64)
    for r in res.results:
        tot += r["partial"].astype(np.float64).sum(axis=0)
    loss = tot[0] / (tot[1] + 1e-6)
    return np.array(loss, dtype=np.float32)
